# revision 11
# baseline (speedup 1.0000x reference)
"""Multi-class DICE loss on 8 Trainium2 NeuronCores.

Reference computation (B=16, C=8, H=W=512):
    onehot = (mask[:,None] == arange(C))        # [B,C,H,W]
    num  = sum(output * onehot, axis=(2,3))     # [B,C]
    den1 = sum(output * output, axis=(2,3))     # [B,C]
    den2 = sum(onehot, axis=(2,3))              # [B,C]
    dice = 2 * (num + eps) / (den1 + den2 + eps)
    loss = 1 - sum(dice) / (B*B)

Sharding: pure data parallel over batch. Each of the 8 cores takes 2
samples, computes per-(b,c) partial sums via fused reduce ops (DVE
is_equal+accum for den2, DVE scalar_tensor_tensor for num, ACT
Square+accum for den1), folds the partition axis with a PE matmul
against ones, computes its local sum of dice, and an AllReduce
produces the global dice sum; every core then emits the final scalar
loss.
"""

import os
from contextlib import ExitStack

import numpy as np

import concourse.bacc as bacc
import concourse.bass as bass
import concourse.tile as tile
from concourse import mybir
from concourse.bass_utils import run_bass_kernel_spmd

N_CORES = 8
B, C, H, W = 16, 8, 512, 512
B_LOC = B // N_CORES          # samples per core
HWPIX = H * W                 # 262144 pixels per (b, c)
P = 128                       # SBUF partitions
NCOL = HWPIX // P             # 2048 free-dim columns per tile
ROWS = B_LOC * C              # 16 (b, c) pairs per core
EPS = 1e-7

_cache: dict = {}
last_results = None           # BassKernelResults of the most recent run


def _emit_rep(nc, pools, x, m, mask64, ones):
    """Emit one full pass over this core's shard; returns the local dice-sum tile."""
    f32 = mybir.dt.float32
    i32 = mybir.dt.int32
    m_cols = NCOL * 2 if mask64 else NCOL
    xpool, mpool, mfpool, spool, acc, pspool = pools

    # Per-partition partial sums, one column per (b, c) pair.
    p_num = acc.tile([P, ROWS], f32, tag="p_num")
    p_den1 = acc.tile([P, ROWS], f32, tag="p_den1")
    p_den2 = acc.tile([P, ROWS], f32, tag="p_den2")

    for b in range(B_LOC):
        mraw = mpool.tile([P, m_cols], i32)
        nc.sync.dma_start(out=mraw, in_=m[b])
        mf = mfpool.tile([P, NCOL], f32)
        if mask64:
            msrc = mraw.rearrange("p (n two) -> p n two", two=2)[:, :, 0]
        else:
            msrc = mraw[:]
        # int32 -> f32 label copy; keeps DVE/ACT free for the hot loop.
        nc.gpsimd.tensor_copy(out=mf, in_=msrc)

        for c in range(C):
            col = b * C + c
            xt = xpool.tile([P, NCOL], f32)
            nc.sync.dma_start(out=xt, in_=x[col])

            # num partial = sum((mask == c) * x) fused on DVE
            njunk = spool.tile([P, NCOL], f32, tag="njunk")
            nc.vector.scalar_tensor_tensor(
                out=njunk,
                in0=mf,
                scalar=float(c),
                in1=xt,
                op0=mybir.AluOpType.is_equal,
                op1=mybir.AluOpType.mult,
                accum_out=p_num[:, col : col + 1],
            )
            # den2 partial = sum(mask == c) fused on DVE
            ejunk = spool.tile([P, NCOL], f32, tag="ejunk")
            nc.vector.tensor_scalar(
                out=ejunk,
                in0=mf,
                scalar1=float(c),
                scalar2=None,
                op0=mybir.AluOpType.is_equal,
                op1=mybir.AluOpType.add,
                accum_out=p_den2[:, col : col + 1],
            )
            # den1 partial = sum(x^2) fused on ACT
            sjunk = spool.tile([P, NCOL], f32, tag="sjunk")
            nc.scalar.activation(
                out=sjunk,
                in_=xt,
                func=mybir.ActivationFunctionType.Square,
                accum_out=p_den1[:, col : col + 1],
            )

    # Fold the 128-partition axis: ones^T @ partials -> psum[1, ROWS]
    ps = pspool.tile([1, 3 * ROWS], f32, tag="ps")
    nc.tensor.matmul(
        out=ps[:, 0:ROWS], lhsT=ones[:], rhs=p_num[:], start=True, stop=True
    )
    nc.tensor.matmul(
        out=ps[:, ROWS : 2 * ROWS], lhsT=ones[:], rhs=p_den1[:], start=True, stop=True
    )
    nc.tensor.matmul(
        out=ps[:, 2 * ROWS :], lhsT=ones[:], rhs=p_den2[:], start=True, stop=True
    )

    # dice = 2 * (num + eps) / (den1 + den2 + eps); S = sum(dice)
    # (PSUM -> SBUF first: TensorTensor may read at most one PSUM input)
    sb = acc.tile([1, 3 * ROWS], f32, tag="sb48")
    nc.scalar.copy(out=sb, in_=ps[:])
    den = acc.tile([1, ROWS], f32, tag="den")
    nc.vector.tensor_add(out=den, in0=sb[:, ROWS : 2 * ROWS], in1=sb[:, 2 * ROWS :])
    dene = acc.tile([1, ROWS], f32, tag="dene")
    nc.vector.tensor_scalar_add(out=dene, in0=den, scalar1=EPS)
    rec = acc.tile([1, ROWS], f32, tag="rec")
    nc.vector.reciprocal(out=rec, in_=dene)
    nume = acc.tile([1, ROWS], f32, tag="nume")
    nc.vector.tensor_scalar_add(out=nume, in0=sb[:, 0:ROWS], scalar1=EPS)
    # local_sum = sum((num+eps) * rec); the dice factor of 2 is folded
    # into the final affine.
    dj = acc.tile([1, ROWS], f32, tag="dj")
    local_sum = acc.tile([1, 1], f32, tag="lsum")
    nc.vector.scalar_tensor_tensor(
        out=dj,
        in0=nume,
        scalar=0.0,
        in1=rec,
        op0=mybir.AluOpType.add,
        op1=mybir.AluOpType.mult,
        accum_out=local_sum,
    )
    return local_sum


def _make_pools(tc, ctx):
    xpool = ctx.enter_context(tc.tile_pool(name="xp", bufs=3))
    mpool = ctx.enter_context(tc.tile_pool(name="mp", bufs=2))
    mfpool = ctx.enter_context(tc.tile_pool(name="mfp", bufs=2))
    spool = ctx.enter_context(tc.tile_pool(name="sp", bufs=2))
    acc = ctx.enter_context(tc.tile_pool(name="acc", bufs=2))
    pspool = ctx.enter_context(tc.tile_pool(name="ps", bufs=2, space="PSUM"))
    return (xpool, mpool, mfpool, spool, acc, pspool)


def _build(mask64: bool, reps: int = 1, collective: bool = True) -> bass.Bass:
    """Build the SPMD Bass program.

    reps > 1 unrolls the whole per-core body multiple times inside one
    NEFF (identical work each rep). collective=False replaces the
    AllReduce epilogue with local-only math (used for single-core
    timeline/cost-model analysis).
    """
    nc = bacc.Bacc(
        "TRN2",
        target_bir_lowering=False,
        debug=False,
        num_devices=N_CORES if collective else 1,
    )
    f32 = mybir.dt.float32
    i32 = mybir.dt.int32

    x = nc.dram_tensor("x", [ROWS, P, NCOL], f32, kind="ExternalInput")
    # int64 masks are fed as little-endian int32 pairs; the low word holds
    # the label (0..7), extracted on-chip with a stride-2 access pattern.
    m_cols = NCOL * 2 if mask64 else NCOL
    m = nc.dram_tensor("m", [B_LOC, P, m_cols], i32, kind="ExternalInput")
    loss = nc.dram_tensor("loss", [1], f32, kind="ExternalOutput")

    with tile.TileContext(nc) as tc, ExitStack() as ctx:
        pools = _make_pools(tc, ctx)
        const = ctx.enter_context(tc.tile_pool(name="const", bufs=1))
        dpool = ctx.enter_context(tc.tile_pool(name="dp", bufs=1, space="DRAM"))
        acc = pools[4]

        ones = const.tile([P, 1], f32)
        nc.vector.memset(ones, 1.0)

        local_sum = None
        for _rep in range(reps):
            local_sum = _emit_rep(nc, pools, x, m, mask64, ones)

        # AllReduce the per-core dice sums, then loss = 1 - 2*S / (B*B)
        if collective:
            cc_in = dpool.tile([1, 1], f32)
            cc_out = dpool.tile([1, 1], f32)
            nc.sync.dma_start(out=cc_in, in_=local_sum)
            nc.gpsimd.collective_compute(
                "AllReduce",
                mybir.AluOpType.add,
                replica_groups=[list(range(N_CORES))],
                ins=[cc_in.opt()],
                outs=[cc_out.opt()],
            )
            global_sum = acc.tile([1, 1], f32, tag="gsum")
            nc.sync.dma_start(out=global_sum, in_=cc_out)
        else:
            global_sum = local_sum
        loss_t = acc.tile([1, 1], f32, tag="loss_t")
        nc.vector.tensor_scalar(
            out=loss_t,
            in0=global_sum,
            scalar1=-2.0 / (B * B),
            scalar2=1.0,
            op0=mybir.AluOpType.mult,
            op1=mybir.AluOpType.add,
        )
        nc.sync.dma_start(out=loss[:], in_=loss_t)

    nc.compile()
    return nc


def _get(mask64: bool) -> bass.Bass:
    if mask64 not in _cache:
        _cache[mask64] = _build(mask64)
    return _cache[mask64]


def make_in_maps(output: np.ndarray, mask: np.ndarray, mask64: bool):
    in_maps = []
    for i in range(N_CORES):
        xs = output[i * B_LOC : (i + 1) * B_LOC].reshape(ROWS, P, NCOL)
        ms = np.ascontiguousarray(mask[i * B_LOC : (i + 1) * B_LOC])
        if mask64:
            ms = ms.view(np.int32).reshape(B_LOC, P, NCOL * 2)
        else:
            ms = ms.reshape(B_LOC, P, NCOL)
        in_maps.append({"x": np.ascontiguousarray(xs), "m": ms})
    return in_maps


def kernel(output: np.ndarray, mask: np.ndarray) -> np.ndarray:
    global last_results
    output = np.ascontiguousarray(np.asarray(output, dtype=np.float32))
    mask = np.asarray(mask)
    assert output.shape == (B, C, H, W), output.shape
    assert mask.shape == (B, H, W), mask.shape
    mask64 = mask.dtype.itemsize == 8
    if not mask64 and mask.dtype != np.int32:
        mask = mask.astype(np.int32)

    nc = _get(mask64)
    in_maps = make_in_maps(output, mask, mask64)
    last_results = run_bass_kernel_spmd(
        nc,
        in_maps,
        list(range(N_CORES)),
        trace=bool(os.environ.get("DICE_TRACE")),
    )
    return np.asarray(last_results.results[0]["loss"], dtype=np.float32).reshape(())
